# revision 5
# baseline (speedup 1.0000x reference)
"""Causal multi-head attention on 8 TRN2 NeuronCores.

Problem: B=4, T=2048, d_model=1024, 16 heads x 64. out = softmax(causal(QK^T)/8) V Wo.

Sharding (tensor-parallel heads x data-parallel batch):
  core c -> batch b = c//2, head group g = c%2 (8 heads each).
  Each core computes a partial output  z_g[b] @ Wo[g] : [2048, 1024];
  host sums the two head-group partials per batch.

Per-core kernel. The PE is the bottleneck (~557k streamed psum columns);
everything is organized to keep it issuing back-to-back (TRN2 PE p-state
reaches full clock only after ~3us of continuous busy):
  - per-HEAD attention pipeline (not head-pairs) so score psum tiles are
    2 banks each and can double-buffer within 8 PSUM banks;
  - software pipeline: scores(i+1) and a few proj/outproj "fill" matmuls
    issue between scores(i) and AV(i), hiding the ACT exp latency;
  - softmax denominator comes free via a ones-column in V; the divide is
    DVE reciprocal_approx_fast on the PE-broadcast denominator (no ACT
    Ln/Exp on the critical path).
"""
import numpy as np

import concourse.bass as bass
import concourse.tile as tile
import concourse.mybir as mybir
from concourse.vector_clock import ScopedClock
from concourse.bass_utils import run_bass_kernel_spmd

D_MODEL = 1024
D_HEAD = 64
B = 4
T = 2048
H = 8              # heads per core
HG = H * D_HEAD    # 512 head-dim columns per core
TCH = 512          # q/t chunk
NCH = T // TCH     # 4
NDM = D_MODEL // 128  # 8 d_model chunks

F32R = mybir.dt.float32r
F32 = mybir.dt.float32
BF16 = mybir.dt.bfloat16
AF = mybir.ActivationFunctionType


class _TC(tile.TileContext):
    """TileContext whose tail drain carries no sem waits (this walrus build
    rejects >1 sync wait per instruction and any wait on a Drain)."""

    def _drain_and_barrier(self, tick_clock, wait_clock):
        drain_inst = self.nc.sync.drain()
        wait_clock.add_sem_waits(
            drain_inst.ins, ScopedClock({None: tick_clock.global_clock})
        )
        si = drain_inst.ins.sync_info
        waits = list(si.on_wait) if si is not None else []
        if waits:
            drain_inst.ins.sync_info = mybir.SyncInfo(
                on_wait=[], on_update=list(si.on_update)
            )
            for w in waits:
                nop = self.nc.sync.nop(nofuse=True)
                nop.ins.sync_info = mybir.SyncInfo(on_wait=[w], on_update=[])
        self.nc.all_engine_barrier()
        popped = self.nc._tile_sem_poison_stack.pop()
        assert popped is self._sem_poison
        self.nc.clear_and_free_semaphores(list(self.sems.allocated().values()))
        self.nc.all_engine_barrier()


def _split_multi_waits(nc):
    """Move all-but-one sem wait of every instruction onto same-engine NOPs."""
    cnt = 0
    for f in nc.m.functions:
        for b in f.blocks:
            new = []
            for inst in b.instructions:
                si = inst.sync_info
                if si is not None and si.on_wait is not None:
                    waits = list(si.on_wait)
                    max_keep = 0 if inst.opcode == "Drain" else 1
                    if len(waits) > max_keep:
                        keep = waits[len(waits) - max_keep:] if max_keep else []
                        spill = waits[: len(waits) - max_keep]
                        for w in spill:
                            nop = mybir.InstNoOp(
                                name=f"I-wsplit-{cnt}", engine=inst.engine,
                                ins=[], outs=[],
                            )
                            nop.sync_info = mybir.SyncInfo(
                                on_wait=[w], on_update=[]
                            )
                            new.append(nop)
                            cnt += 1
                        inst.sync_info = mybir.SyncInfo(
                            on_wait=keep, on_update=list(si.on_update)
                        )
                new.append(inst)
            b.instructions = new
    return cnt


def _build():
    nc = bass.Bass("TRN2", target_bir_lowering=False)
    xT = nc.dram_tensor("xT", (D_MODEL, T), F32R, kind="ExternalInput")
    wq = nc.dram_tensor("wq", (D_MODEL, HG), F32R, kind="ExternalInput")
    wk = nc.dram_tensor("wk", (D_MODEL, HG), F32R, kind="ExternalInput")
    wv = nc.dram_tensor("wv", (D_MODEL, HG), F32R, kind="ExternalInput")
    wo = nc.dram_tensor("wo", (HG, D_MODEL), F32R, kind="ExternalInput")
    tri = nc.dram_tensor("tri", (128, 128), BF16, kind="ExternalInput")
    ones1 = nc.dram_tensor("ones1", (1, 64), F32R, kind="ExternalInput")
    vones = nc.dram_tensor("vones", (128, T // 128, H, 1), BF16,
                           kind="ExternalInput")
    out = nc.dram_tensor("out", (T, D_MODEL), F32, kind="ExternalOutput")

    from contextlib import ExitStack
    with _TC(nc) as tc, ExitStack() as ctx:
        consts = ctx.enter_context(tc.tile_pool(name="consts", bufs=1))
        xs_pool = ctx.enter_context(tc.tile_pool(name="xs", bufs=2))
        kt_pool = ctx.enter_context(tc.tile_pool(name="kt", bufs=1))
        v_pool = ctx.enter_context(tc.tile_pool(name="v", bufs=1))
        qt_pool = ctx.enter_context(tc.tile_pool(name="qt", bufs=2))
        zt_pool = ctx.enter_context(tc.tile_pool(name="zt", bufs=2))
        et_pool = ctx.enter_context(tc.tile_pool(name="et", bufs=3))
        sm_pool = ctx.enter_context(tc.tile_pool(name="sm", bufs=2))
        rb_pool = ctx.enter_context(tc.tile_pool(name="rb", bufs=2))
        ou_pool = ctx.enter_context(tc.tile_pool(name="ou", bufs=2))
        # PSUM: 8 banks total. s2 tiles are [128,2,512]f32 = 2 banks x 2 bufs;
        # u tiles 1 bank x 2; w tiles (proj/outproj/bcast share one tag) 1x2.
        ps_s = ctx.enter_context(tc.tile_pool(name="ps_s", bufs=2, space="PSUM"))
        ps_u = ctx.enter_context(tc.tile_pool(name="ps_u", bufs=2, space="PSUM"))
        ps_w = ctx.enter_context(tc.tile_pool(name="ps_w", bufs=2, space="PSUM"))

        xT_r = xT.ap().rearrange("(c p) t -> p c t", p=128)

        # resident weights / constants (wq/wk + first x chunk lead: they gate
        # the first matmuls)
        wq_sb = consts.tile([128, NDM, HG], F32R)
        xs0 = xs_pool.tile([128, NDM, TCH], F32R, name="xs", tag="xs")
        wk_sb = consts.tile([128, NDM, HG], F32R)
        wv_sb = consts.tile([128, NDM, HG], F32R)
        wq_r = wq.ap().rearrange("(c p) n -> p c n", p=128)
        wk_r = wk.ap().rearrange("(c p) n -> p c n", p=128)
        for c in range(NDM):
            nc.sync.dma_start(out=wq_sb[:, c, :], in_=wq_r[:, c, :])
            nc.sync.dma_start(out=wk_sb[:, c, :], in_=wk_r[:, c, :])
            nc.sync.dma_start(out=xs0[:, c, :], in_=xT_r[:, c, 0:TCH])
        nc.sync.dma_start(out=wv_sb, in_=wv.ap().rearrange("(c p) n -> p c n", p=128))
        tri_sb = consts.tile([128, 128], BF16)
        nc.sync.dma_start(out=tri_sb, in_=tri.ap())
        ones_sb = consts.tile([1, 64], F32R)
        nc.sync.dma_start(out=ones_sb, in_=ones1.ap())
        wo_sb = consts.tile([128, HG // 128, D_MODEL], F32R)
        nc.sync.dma_start(out=wo_sb, in_=wo.ap().rearrange("(c p) n -> p c n", p=128))
        # per-chunk K^T tiles [pair-packed 128, pair, t-in-chunk] and V tiles
        # (V has a ones column so row 64 of U accumulates the denominator)
        kt_tiles = [kt_pool.tile([128, 4, TCH], BF16, name=f"kt{i}", tag=f"kt{i}")
                    for i in range(NCH)]
        v_tiles = [v_pool.tile([128, 4, H, D_HEAD + 1], BF16, name=f"v{i}",
                               tag=f"v{i}") for i in range(NCH)]
        vo_r = vones.ap().rearrange("p (a b) h o -> p a b h o", b=4)
        for i in range(NCH):
            nc.sync.dma_start(out=v_tiles[i][:, :, :, D_HEAD:], in_=vo_r[:, i])

        # ---- fill atoms: proj / outproj work split into single-PE-op units
        def proj_atoms(ch, xs, qt_sb):
            """Q^T,K^T per dqc and V per tt for chunk ch. Each atom = 1 PE mm
            or 1 DVE copy."""
            atoms = []
            state = {}
            for dqc in range(4):
                def a_q(c, dqc=dqc):
                    if c == 0:
                        state[('q', dqc)] = ps_w.tile([128, TCH], F32, tag="w",
                                                      name="pq")
                    nc.tensor.matmul(
                        state[('q', dqc)],
                        lhsT=wq_sb[:, c, dqc * 128:(dqc + 1) * 128],
                        rhs=xs[:, c, :], start=(c == 0), stop=(c == NDM - 1))
                for c in range(NDM):
                    atoms.append(lambda c=c, a=a_q: a(c))
                atoms.append(lambda dqc=dqc: nc.vector.tensor_copy(
                    out=qt_sb[:, dqc, :], in_=state[('q', dqc)]))
                def a_k(c, dqc=dqc):
                    if c == 0:
                        state[('k', dqc)] = ps_w.tile([128, TCH], F32, tag="w",
                                                      name="pk")
                    nc.tensor.matmul(
                        state[('k', dqc)],
                        lhsT=wk_sb[:, c, dqc * 128:(dqc + 1) * 128],
                        rhs=xs[:, c, :], start=(c == 0), stop=(c == NDM - 1))
                for c in range(NDM):
                    atoms.append(lambda c=c, a=a_k: a(c))
                atoms.append(lambda dqc=dqc: nc.vector.tensor_copy(
                    out=kt_tiles[ch][:, dqc, :], in_=state[('k', dqc)]))
            for tt in range(4):
                def a_v(c, tt=tt):
                    if c == 0:
                        state[('v', tt)] = ps_w.tile([128, HG], F32, tag="w",
                                                     name="pv")
                    nc.tensor.matmul(
                        state[('v', tt)],
                        lhsT=xs[:, c, tt * 128:(tt + 1) * 128],
                        rhs=wv_sb[:, c, :], start=(c == 0), stop=(c == NDM - 1))
                for c in range(NDM):
                    atoms.append(lambda c=c, a=a_v: a(c))
                atoms.append(lambda tt=tt: nc.vector.tensor_copy(
                    out=v_tiles[ch][:, tt, :, 0:D_HEAD],
                    in_=state[('v', tt)].rearrange("p (h d) -> p h d", h=H)))
            return atoms

        def outproj_atoms(ch, zt_sb):
            atoms = []
            q0 = ch * TCH
            state = {}
            for tt in range(4):
                def a_alloc(tt=tt):
                    state[('o', tt)] = ou_pool.tile([128, D_MODEL], F32,
                                                    name="o_sb", tag="o")
                atoms.append(a_alloc)
                for dc in range(2):
                    def a_mm(kc, tt=tt, dc=dc):
                        if kc == 0:
                            state[('p', tt, dc)] = ps_w.tile(
                                [128, 512], F32, tag="w", name="po")
                        nc.tensor.matmul(
                            state[('p', tt, dc)],
                            lhsT=zt_sb[:, kc, tt * 128:(tt + 1) * 128],
                            rhs=wo_sb[:, kc, dc * 512:(dc + 1) * 512],
                            start=(kc == 0), stop=(kc == 3))
                    for kc in range(4):
                        atoms.append(lambda kc=kc, a=a_mm: a(kc))
                    atoms.append(lambda tt=tt, dc=dc: nc.vector.tensor_copy(
                        out=state[('o', tt)][:, dc * 512:(dc + 1) * 512],
                        in_=state[('p', tt, dc)]))
                def a_dma(tt=tt):
                    r0 = q0 + tt * 128
                    nc.sync.dma_start(out=out.ap()[r0:r0 + 128, :],
                                      in_=state[('o', tt)])
                atoms.append(a_dma)
            return atoms

        # ---- attention emission for one chunk, fills interleaved ----
        qt_tiles = [None] * NCH
        xs_tiles = [xs0] + [None] * (NCH - 1)
        zt_tiles = [None] * NCH

        def dma_xs(ch):
            xs_tiles[ch] = xs_pool.tile([128, NDM, TCH], F32R, name="xs",
                                        tag="xs")
            nc.sync.dma_start(out=xs_tiles[ch],
                              in_=xT_r[:, :, ch * TCH:(ch + 1) * TCH])

        def attention_chunk(ch, fills):
            nkb = 4 * ch + 4
            nkb2 = nkb // 2
            qt_sb = qt_tiles[ch]
            zt_sb = zt_tiles[ch]
            st = {}

            def emit_S(h, kb2):
                hp, p0 = h // 2, 64 * (h % 2)
                kba, kbb = 2 * kb2, 2 * kb2 + 1
                ja, jb = kba - 4 * ch, kbb - 4 * ch
                ca = 128 * ja if ja > 0 else 0
                cb = 128 * jb if jb > 0 else 0
                oa, ob = (kba % 4) * 128, (kbb % 4) * 128
                s2 = ps_s.tile([128, 2, TCH], F32, tag="s2", name="s2")
                nc.tensor.matmul(
                    s2[:, 0, ca:],
                    lhsT=kt_tiles[kba // 4][p0:p0 + 64, hp, oa:oa + 128],
                    rhs=qt_sb[p0:p0 + 64, hp, ca:],
                    start=True, stop=True, tile_position=(p0, 0))
                nc.tensor.matmul(
                    s2[:, 1, cb:],
                    lhsT=kt_tiles[kbb // 4][p0:p0 + 64, hp, ob:ob + 128],
                    rhs=qt_sb[p0:p0 + 64, hp, cb:],
                    start=True, stop=True, tile_position=(p0, 0))
                et = et_pool.tile([128, 2, TCH], BF16, name="et", tag="et")
                s2f = s2.rearrange("p a b -> p (a b)")
                etf = et.rearrange("p a b -> p (a b)")
                nc.scalar.activation(out=etf[:, ca:], in_=s2f[:, ca:],
                                     func=AF.Exp, scale=0.125)
                if ja >= 0:
                    nc.vector.tensor_mul(et[:, 0, ca:ca + 128],
                                         et[:, 0, ca:ca + 128], tri_sb)
                if jb >= 0:
                    nc.vector.tensor_mul(et[:, 1, cb:cb + 128],
                                         et[:, 1, cb:cb + 128], tri_sb)
                st[(h, kb2)] = et

            def emit_A(h, kb2):
                et = st.pop((h, kb2))
                kba, kbb = 2 * kb2, 2 * kb2 + 1
                ja, jb = kba - 4 * ch, kbb - 4 * ch
                ca = 128 * ja if ja > 0 else 0
                cb = 128 * jb if jb > 0 else 0
                u = st[('u', h)]
                nc.tensor.matmul(
                    u[:, ca:], lhsT=v_tiles[kba // 4][:, kba % 4, h, :],
                    rhs=et[:, 0, ca:], start=(kba == 0), stop=False)
                nc.tensor.matmul(
                    u[:, cb:], lhsT=v_tiles[kbb // 4][:, kbb % 4, h, :],
                    rhs=et[:, 1, cb:], start=False, stop=(kbb == nkb - 1))

            def emit_divA(h):
                # 1/denominator row -> SBUF f32r (bcast matmul rhs); native
                # DVE reciprocal straight off the psum row.
                u = st[('u', h)]
                rcp = sm_pool.tile([1, TCH], F32R, name="rcp", tag="rcp")
                with nc.allow_low_precision(reason="f32r is fp32 bits"):
                    nc.vector.reciprocal(out=rcp, in_=u[D_HEAD:D_HEAD + 1, :])
                st[('d', h)] = rcp

            def emit_divB(h):
                hp, p0 = h // 2, 64 * (h % 2)
                u = st.pop(('u', h))
                rcp = st.pop(('d', h))
                db = ps_w.tile([64, TCH], F32, tag="w", name="db")
                nc.tensor.matmul(db, lhsT=ones_sb, rhs=rcp,
                                 start=True, stop=True)
                rb = rb_pool.tile([64, TCH], F32, name="rb", tag="rb")
                nc.vector.tensor_copy(out=rb, in_=db)
                nc.vector.tensor_mul(zt_sb[p0:p0 + 64, hp, :],
                                     u[0:D_HEAD, :], rb)

            order = [(h, kb2) for h in range(H) for kb2 in range(nkb2)]
            n = len(order)
            F = len(fills)
            fi = 0
            pend_A = None
            pend_div = []
            for idx, (h, kb2) in enumerate(order):
                if kb2 == 0:
                    st[('u', h)] = ps_u.tile([D_HEAD + 1, TCH], F32, name="u",
                                             tag="u")
                emit_S(h, kb2)
                want = (idx + 1) * F // n
                while fi < want:
                    fills[fi]()
                    fi += 1
                # divB of a finished head goes after the next S+fills so its
                # bcast matmul doesn't make the PE wait on the DVE D-copy.
                while pend_div:
                    emit_divB(pend_div.pop(0))
                if pend_A is not None:
                    emit_A(*pend_A)
                    if pend_A[1] == nkb2 - 1:
                        emit_divA(pend_A[0])
                        pend_div.append(pend_A[0])
                pend_A = (h, kb2)
            emit_A(*pend_A)
            emit_divA(pend_A[0])
            while fi < F:
                fills[fi]()
                fi += 1
            emit_divB(pend_A[0])

        # ---- schedule ----
        # proj(0) upfront; per chunk ch: fills = outproj(ch-1) + proj(ch+1).
        dma_xs(1)
        qt_tiles[0] = qt_pool.tile([128, 4, TCH], BF16, name="qt", tag="qt")
        for a in proj_atoms(0, xs_tiles[0], qt_tiles[0]):
            a()
        for ch in range(NCH):
            zt_tiles[ch] = zt_pool.tile([128, 4, TCH], F32R, name="zt",
                                        tag="zt")
            fills = []
            if ch >= 1:
                fills += outproj_atoms(ch - 1, zt_tiles[ch - 1])
            if ch + 1 < NCH:
                if ch + 2 < NCH:
                    dma_xs(ch + 2)
                qt_tiles[ch + 1] = qt_pool.tile([128, 4, TCH], BF16,
                                                name="qt", tag="qt")
                fills += proj_atoms(ch + 1, xs_tiles[ch + 1],
                                    qt_tiles[ch + 1])
            attention_chunk(ch, fills)
        for a in outproj_atoms(NCH - 1, zt_tiles[NCH - 1]):
            a()

    _split_multi_waits(nc)
    return nc


_NC_CACHE = None


def _get_nc():
    global _NC_CACHE
    if _NC_CACHE is None:
        _NC_CACHE = _build()
    return _NC_CACHE


def _make_in_maps(x, W_Q, W_K, W_V, W_O):
    x = np.asarray(x, dtype=np.float32)
    W_Q = np.asarray(W_Q, dtype=np.float32)
    W_K = np.asarray(W_K, dtype=np.float32)
    W_V = np.asarray(W_V, dtype=np.float32)
    W_O = np.asarray(W_O, dtype=np.float32)

    import ml_dtypes
    tri = np.triu(np.ones((128, 128), dtype=ml_dtypes.bfloat16))  # col >= row
    ones1 = np.ones((1, 64), dtype=np.float32)
    vones = np.ones((128, T // 128, H, 1), dtype=ml_dtypes.bfloat16)

    in_maps = []
    for core in range(8):
        b, g = core // 2, core % 2
        cs = slice(g * HG, (g + 1) * HG)
        in_maps.append({
            "xT": np.ascontiguousarray(x[b].T),
            "wq": np.ascontiguousarray(W_Q[:, cs]),
            "wk": np.ascontiguousarray(W_K[:, cs]),
            "wv": np.ascontiguousarray(W_V[:, cs]),
            "wo": np.ascontiguousarray(W_O[cs, :]),
            "tri": tri, "ones1": ones1, "vones": vones,
        })
    return in_maps


def kernel(x, W_Q, W_K, W_V, W_O):
    in_maps = _make_in_maps(x, W_Q, W_K, W_V, W_O)
    nc = _get_nc()
    res = run_bass_kernel_spmd(nc, in_maps, core_ids=list(range(8)))
    outs = [res.results[c]["out"] for c in range(8)]
    full = np.stack([outs[2 * b] + outs[2 * b + 1] for b in range(B)], axis=0)
    return full


# revision 10
# speedup vs baseline: 1.2304x; 1.2304x over previous
"""Causal multi-head attention on 8 TRN2 NeuronCores.

Problem: B=4, T=2048, d_model=1024, 16 heads x 64. out = softmax(causal(QK^T)/8) V Wo.

Sharding (tensor-parallel heads x data-parallel batch):
  core c -> batch b = c//2, head group g = c%2 (8 heads each).
  Each core computes a partial output  z_g[b] @ Wo[g] : [2048, 1024];
  host sums the two head-group partials per batch.

Per-core kernel. The PE is the bottleneck (~557k streamed psum columns);
everything is organized to keep it issuing back-to-back (TRN2 PE p-state
reaches full clock only after ~3us of continuous busy):
  - per-HEAD attention pipeline (not head-pairs) so score psum tiles are
    2 banks each and can double-buffer within 8 PSUM banks;
  - software pipeline: scores(i+1) and a few proj/outproj "fill" matmuls
    issue between scores(i) and AV(i), hiding the ACT exp latency;
  - softmax denominator comes free via a ones-column in V; the divide is
    DVE reciprocal_approx_fast on the PE-broadcast denominator (no ACT
    Ln/Exp on the critical path).
"""
import numpy as np

import concourse.bass as bass
import concourse.tile as tile
import concourse.mybir as mybir
from concourse.vector_clock import ScopedClock
from concourse.bass_utils import run_bass_kernel_spmd

D_MODEL = 1024
D_HEAD = 64
B = 4
T = 2048
H = 8              # heads per core
HG = H * D_HEAD    # 512 head-dim columns per core
TCH = 512          # q/t chunk
NCH = T // TCH     # 4
NDM = D_MODEL // 128  # 8 d_model chunks

F32R = mybir.dt.float32r
F32 = mybir.dt.float32
BF16 = mybir.dt.bfloat16
AF = mybir.ActivationFunctionType


class _TC(tile.TileContext):
    """TileContext whose tail drain carries no sem waits (this walrus build
    rejects >1 sync wait per instruction and any wait on a Drain)."""

    def _drain_and_barrier(self, tick_clock, wait_clock):
        drain_inst = self.nc.sync.drain()
        wait_clock.add_sem_waits(
            drain_inst.ins, ScopedClock({None: tick_clock.global_clock})
        )
        si = drain_inst.ins.sync_info
        waits = list(si.on_wait) if si is not None else []
        if waits:
            drain_inst.ins.sync_info = mybir.SyncInfo(
                on_wait=[], on_update=list(si.on_update)
            )
            for w in waits:
                nop = self.nc.sync.nop(nofuse=True)
                nop.ins.sync_info = mybir.SyncInfo(on_wait=[w], on_update=[])
        self.nc.all_engine_barrier()
        popped = self.nc._tile_sem_poison_stack.pop()
        assert popped is self._sem_poison
        self.nc.clear_and_free_semaphores(list(self.sems.allocated().values()))
        self.nc.all_engine_barrier()


def _split_multi_waits(nc):
    """Move all-but-one sem wait of every instruction onto same-engine NOPs."""
    cnt = 0
    for f in nc.m.functions:
        for b in f.blocks:
            new = []
            for inst in b.instructions:
                si = inst.sync_info
                if si is not None and si.on_wait is not None:
                    waits = list(si.on_wait)
                    max_keep = 0 if inst.opcode == "Drain" else 1
                    if len(waits) > max_keep:
                        keep = waits[len(waits) - max_keep:] if max_keep else []
                        spill = waits[: len(waits) - max_keep]
                        for w in spill:
                            nop = mybir.InstNoOp(
                                name=f"I-wsplit-{cnt}", engine=inst.engine,
                                ins=[], outs=[],
                            )
                            nop.sync_info = mybir.SyncInfo(
                                on_wait=[w], on_update=[]
                            )
                            new.append(nop)
                            cnt += 1
                        inst.sync_info = mybir.SyncInfo(
                            on_wait=keep, on_update=list(si.on_update)
                        )
                new.append(inst)
            b.instructions = new
    return cnt


def _build():
    nc = bass.Bass("TRN2", target_bir_lowering=False)
    xT = nc.dram_tensor("xT", (D_MODEL, T), F32R, kind="ExternalInput")
    wq = nc.dram_tensor("wq", (D_MODEL, HG), F32R, kind="ExternalInput")
    wk = nc.dram_tensor("wk", (D_MODEL, HG), F32R, kind="ExternalInput")
    wv = nc.dram_tensor("wv", (D_MODEL, HG), F32R, kind="ExternalInput")
    wo = nc.dram_tensor("wo", (HG, D_MODEL), F32R, kind="ExternalInput")
    tri = nc.dram_tensor("tri", (128, 128), BF16, kind="ExternalInput")
    ones1 = nc.dram_tensor("ones1", (1, 64), F32R, kind="ExternalInput")
    vones = nc.dram_tensor("vones", (128, T // 128, H, 1), BF16,
                           kind="ExternalInput")
    out = nc.dram_tensor("out", (T, D_MODEL), F32, kind="ExternalOutput")

    from contextlib import ExitStack
    with _TC(nc) as tc, ExitStack() as ctx:
        consts = ctx.enter_context(tc.tile_pool(name="consts", bufs=1))
        xs_pool = ctx.enter_context(tc.tile_pool(name="xs", bufs=2))
        kt_pool = ctx.enter_context(tc.tile_pool(name="kt", bufs=1))
        v_pool = ctx.enter_context(tc.tile_pool(name="v", bufs=1))
        qt_pool = ctx.enter_context(tc.tile_pool(name="qt", bufs=2))
        zt_pool = ctx.enter_context(tc.tile_pool(name="zt", bufs=2))
        et_pool = ctx.enter_context(tc.tile_pool(name="et", bufs=3))
        sm_pool = ctx.enter_context(tc.tile_pool(name="sm", bufs=2))
        rb_pool = ctx.enter_context(tc.tile_pool(name="rb", bufs=2))
        ou_pool = ctx.enter_context(tc.tile_pool(name="ou", bufs=2))
        # PSUM: 8 banks total. s2 tiles are [128,2,512]f32 = 2 banks x 2 bufs;
        # u tiles 1 bank x 2; w tiles (proj/outproj/bcast share one tag) 1x2.
        ps_s = ctx.enter_context(tc.tile_pool(name="ps_s", bufs=2, space="PSUM"))
        ps_u = ctx.enter_context(tc.tile_pool(name="ps_u", bufs=2, space="PSUM"))
        ps_w = ctx.enter_context(tc.tile_pool(name="ps_w", bufs=2, space="PSUM"))

        xT_r = xT.ap().rearrange("(c p) t -> p c t", p=128)

        # resident weights / constants (wq/wk + first x chunk lead: they gate
        # the first matmuls)
        wq_sb = consts.tile([128, NDM, HG], F32R)
        xs0 = xs_pool.tile([128, NDM, TCH], F32R, name="xs", tag="xs")
        wk_sb = consts.tile([128, NDM, HG], F32R)
        wv_sb = consts.tile([128, NDM, HG], F32R)
        wq_r = wq.ap().rearrange("(c p) n -> p c n", p=128)
        wk_r = wk.ap().rearrange("(c p) n -> p c n", p=128)
        for c in range(NDM):
            nc.sync.dma_start(out=wq_sb[:, c, :], in_=wq_r[:, c, :])
            nc.sync.dma_start(out=wk_sb[:, c, :], in_=wk_r[:, c, :])
            nc.sync.dma_start(out=xs0[:, c, :], in_=xT_r[:, c, 0:TCH])
        nc.sync.dma_start(out=wv_sb, in_=wv.ap().rearrange("(c p) n -> p c n", p=128))
        tri_sb = consts.tile([128, 128], BF16)
        nc.sync.dma_start(out=tri_sb, in_=tri.ap())
        ones_sb = consts.tile([1, 64], F32R)
        nc.sync.dma_start(out=ones_sb, in_=ones1.ap())
        wo_sb = consts.tile([128, HG // 128, D_MODEL], F32R)
        nc.sync.dma_start(out=wo_sb, in_=wo.ap().rearrange("(c p) n -> p c n", p=128))
        # per-chunk K^T tiles [pair-packed 128, pair, t-in-chunk] and V tiles
        # (V has a ones column so row 64 of U accumulates the denominator)
        kt_tiles = [kt_pool.tile([128, 4, TCH], BF16, name=f"kt{i}", tag=f"kt{i}")
                    for i in range(NCH)]
        v_tiles = [v_pool.tile([128, 4, H, D_HEAD + 1], BF16, name=f"v{i}",
                               tag=f"v{i}") for i in range(NCH)]
        vo_r = vones.ap().rearrange("p (a b) h o -> p a b h o", b=4)
        for i in range(NCH):
            nc.sync.dma_start(out=v_tiles[i][:, :, :, D_HEAD:], in_=vo_r[:, i])

        # ---- fill atoms: proj / outproj work split into single-PE-op units
        def proj_parts(ch, xs, qt_sb):
            """Q^T,K^T per dqc and V per tt for chunk ch; returns per-unit
            atom lists. Each atom = 1 PE mm or 1 DVE copy."""
            state = {}
            uq, uk, uv = [], [], []
            for dqc in range(4):
                unit = []
                def a_q(c, dqc=dqc):
                    if c == 0:
                        state[('q', dqc)] = ps_w.tile([128, TCH], F32, tag="w",
                                                      name="pq")
                    nc.tensor.matmul(
                        state[('q', dqc)],
                        lhsT=wq_sb[:, c, dqc * 128:(dqc + 1) * 128],
                        rhs=xs[:, c, :], start=(c == 0), stop=(c == NDM - 1))
                for c in range(NDM):
                    unit.append(lambda c=c, a=a_q: a(c))
                unit.append(lambda dqc=dqc: nc.vector.tensor_copy(
                    out=qt_sb[:, dqc, :], in_=state[('q', dqc)]))
                uq.append(unit)
                unit = []
                def a_k(c, dqc=dqc):
                    if c == 0:
                        state[('k', dqc)] = ps_w.tile([128, TCH], F32, tag="w",
                                                      name="pk")
                    nc.tensor.matmul(
                        state[('k', dqc)],
                        lhsT=wk_sb[:, c, dqc * 128:(dqc + 1) * 128],
                        rhs=xs[:, c, :], start=(c == 0), stop=(c == NDM - 1))
                for c in range(NDM):
                    unit.append(lambda c=c, a=a_k: a(c))
                unit.append(lambda dqc=dqc: nc.vector.tensor_copy(
                    out=kt_tiles[ch][:, dqc, :], in_=state[('k', dqc)]))
                uk.append(unit)
            for tt in range(4):
                unit = []
                def a_v(c, tt=tt):
                    if c == 0:
                        state[('v', tt)] = ps_w.tile([128, HG], F32, tag="w",
                                                     name="pv")
                    nc.tensor.matmul(
                        state[('v', tt)],
                        lhsT=xs[:, c, tt * 128:(tt + 1) * 128],
                        rhs=wv_sb[:, c, :], start=(c == 0), stop=(c == NDM - 1))
                for c in range(NDM):
                    unit.append(lambda c=c, a=a_v: a(c))
                unit.append(lambda tt=tt: nc.vector.tensor_copy(
                    out=v_tiles[ch][:, tt, :, 0:D_HEAD],
                    in_=state[('v', tt)].rearrange("p (h d) -> p h d", h=H)))
                uv.append(unit)
            return uq, uk, uv

        def outproj_atoms(ch, zt_sb):
            atoms = []
            q0 = ch * TCH
            state = {}
            for tt in range(4):
                def a_alloc(tt=tt):
                    state[('o', tt)] = ou_pool.tile([128, D_MODEL], F32,
                                                    name="o_sb", tag="o")
                atoms.append(a_alloc)
                for dc in range(2):
                    def a_mm(kc, tt=tt, dc=dc):
                        if kc == 0:
                            state[('p', tt, dc)] = ps_w.tile(
                                [128, 512], F32, tag="w", name="po")
                        nc.tensor.matmul(
                            state[('p', tt, dc)],
                            lhsT=zt_sb[:, kc, tt * 128:(tt + 1) * 128],
                            rhs=wo_sb[:, kc, dc * 512:(dc + 1) * 512],
                            start=(kc == 0), stop=(kc == 3))
                    for kc in range(4):
                        atoms.append(lambda kc=kc, a=a_mm: a(kc))
                    atoms.append(lambda tt=tt, dc=dc: nc.vector.tensor_copy(
                        out=state[('o', tt)][:, dc * 512:(dc + 1) * 512],
                        in_=state[('p', tt, dc)]))
                def a_dma(tt=tt):
                    r0 = q0 + tt * 128
                    nc.sync.dma_start(out=out.ap()[r0:r0 + 128, :],
                                      in_=state[('o', tt)])
                atoms.append(a_dma)
            return atoms

        # ---- attention emission for one chunk, fills interleaved ----
        qt_tiles = [None] * NCH
        xs_tiles = [xs0] + [None] * (NCH - 1)
        zt_tiles = [None] * NCH

        def dma_xs(ch):
            xs_tiles[ch] = xs_pool.tile([128, NDM, TCH], F32R, name="xs",
                                        tag="xs")
            for c in range(NDM):
                nc.sync.dma_start(out=xs_tiles[ch][:, c, :],
                                  in_=xT_r[:, c, ch * TCH:(ch + 1) * TCH])

        def attention_chunk(ch, fills):
            nkb = 4 * ch + 4
            nkb2 = nkb // 2
            qt_sb = qt_tiles[ch]
            zt_sb = zt_tiles[ch]
            st = {}

            def emit_S(h, kb2):
                hp, p0 = h // 2, 64 * (h % 2)
                kba, kbb = 2 * kb2, 2 * kb2 + 1
                ja, jb = kba - 4 * ch, kbb - 4 * ch
                ca = 128 * ja if ja > 0 else 0
                cb = 128 * jb if jb > 0 else 0
                oa, ob = (kba % 4) * 128, (kbb % 4) * 128
                s2 = ps_s.tile([128, 2, TCH], F32, tag="s2", name="s2")
                nc.tensor.matmul(
                    s2[:, 0, ca:],
                    lhsT=kt_tiles[kba // 4][p0:p0 + 64, hp, oa:oa + 128],
                    rhs=qt_sb[p0:p0 + 64, hp, ca:],
                    start=True, stop=True, tile_position=(p0, 0))
                nc.tensor.matmul(
                    s2[:, 1, cb:],
                    lhsT=kt_tiles[kbb // 4][p0:p0 + 64, hp, ob:ob + 128],
                    rhs=qt_sb[p0:p0 + 64, hp, cb:],
                    start=True, stop=True, tile_position=(p0, 0))
                et = et_pool.tile([128, 2, TCH], BF16, name="et", tag="et")
                s2f = s2.rearrange("p a b -> p (a b)")
                etf = et.rearrange("p a b -> p (a b)")
                nc.scalar.activation(out=etf[:, ca:], in_=s2f[:, ca:],
                                     func=AF.Exp, scale=0.125)
                if ja >= 0:
                    nc.vector.tensor_mul(et[:, 0, ca:ca + 128],
                                         et[:, 0, ca:ca + 128], tri_sb)
                if jb >= 0:
                    nc.vector.tensor_mul(et[:, 1, cb:cb + 128],
                                         et[:, 1, cb:cb + 128], tri_sb)
                st[(h, kb2)] = et

            def emit_A(h, kb2):
                et = st.pop((h, kb2))
                kba, kbb = 2 * kb2, 2 * kb2 + 1
                ja, jb = kba - 4 * ch, kbb - 4 * ch
                ca = 128 * ja if ja > 0 else 0
                cb = 128 * jb if jb > 0 else 0
                u = st[('u', h)]
                nc.tensor.matmul(
                    u[:, ca:], lhsT=v_tiles[kba // 4][:, kba % 4, h, :],
                    rhs=et[:, 0, ca:], start=(kba == 0), stop=False)
                nc.tensor.matmul(
                    u[:, cb:], lhsT=v_tiles[kbb // 4][:, kbb % 4, h, :],
                    rhs=et[:, 1, cb:], start=False, stop=(kbb == nkb - 1))

            def emit_divA(h):
                # denominator row -> SBUF (cheap; the PE bcast waits only on
                # this, the expensive reciprocal runs after the bcast)
                u = st[('u', h)]
                dsb = sm_pool.tile([1, TCH], F32R, name="dsb", tag="dsb")
                nc.vector.tensor_copy(out=dsb, in_=u[D_HEAD:D_HEAD + 1, :])
                st[('d', h)] = dsb

            def emit_divB(h):
                hp, p0 = h // 2, 64 * (h % 2)
                u = st.pop(('u', h))
                dsb = st.pop(('d', h))
                db = ps_w.tile([64, TCH], F32, tag="w", name="db")
                nc.tensor.matmul(db, lhsT=ones_sb, rhs=dsb,
                                 start=True, stop=True)
                rb = rb_pool.tile([64, TCH], F32, name="rb", tag="rb")
                nc.vector.reciprocal(out=rb, in_=db)
                nc.vector.tensor_mul(zt_sb[p0:p0 + 64, hp, :],
                                     u[0:D_HEAD, :], rb)

            order = [(h, kb2) for h in range(H) for kb2 in range(nkb2)]
            n = len(order)
            # fills: (front, rate) emitted at fixed rate from window 0; rest
            # paced uniformly over the whole chunk.
            front, frate, rest = fills
            Ff, Fr = len(front), len(rest)
            ffi = fi = 0
            pend_A = None
            pend_div = []
            for idx, (h, kb2) in enumerate(order):
                if kb2 == 0:
                    st[('u', h)] = ps_u.tile([D_HEAD + 1, TCH], F32, name="u",
                                             tag="u")
                emit_S(h, kb2)
                wantf = min(Ff, (idx + 1) * frate)
                while ffi < wantf:
                    front[ffi]()
                    ffi += 1
                want = (idx + 1) * Fr // n
                while fi < want:
                    fills[2][fi]()
                    fi += 1
                # divB of a finished head is deferred ~3 windows so its bcast
                # matmul doesn't make the PE wait on the DVE D-copy.
                while pend_div and pend_div[0][1] <= idx:
                    emit_divB(pend_div.pop(0)[0])
                if pend_A is not None:
                    emit_A(*pend_A)
                    if pend_A[1] == nkb2 - 1:
                        emit_divA(pend_A[0])
                        pend_div.append((pend_A[0], idx + 3))
                pend_A = (h, kb2)
            emit_A(*pend_A)
            emit_divA(pend_A[0])
            while ffi < Ff:
                front[ffi]()
                ffi += 1
            while fi < Fr:
                rest[fi]()
                fi += 1
            while pend_div:
                emit_divB(pend_div.pop(0)[0])
            emit_divB(pend_A[0])

        # ---- schedule ----
        # proj(0) upfront. Fills: ch0 <- proj(1); ch1 <- outproj(0)+proj(2);
        # ch2 <- outproj(1)+proj(3).uq; ch3 <- front-loaded proj(3).uk/uv
        # (kt/v of the diagonal chunk, needed from kb2=6) + outproj(2).
        def flat(units):
            return [a for unit in units for a in unit]

        dma_xs(1)
        qt_tiles[0] = qt_pool.tile([128, 4, TCH], BF16, name="qt", tag="qt")
        uq0, uk0, uv0 = proj_parts(0, xs_tiles[0], qt_tiles[0])
        for a in flat([uq0[0], uk0[0], uq0[1], uk0[1], uq0[2], uk0[2],
                       uq0[3], uk0[3]] + uv0):
            a()
        ukv3 = None
        for ch in range(NCH):
            zt_tiles[ch] = zt_pool.tile([128, 4, TCH], F32R, name="zt",
                                        tag="zt")
            front, frate, rest = [], 0, []
            if ch >= 1:
                rest += outproj_atoms(ch - 1, zt_tiles[ch - 1])
            if ch + 1 < NCH:
                if ch + 2 < NCH:
                    dma_xs(ch + 2)
                qt_tiles[ch + 1] = qt_pool.tile([128, 4, TCH], BF16,
                                                name="qt", tag="qt")
                uq, uk, uv = proj_parts(ch + 1, xs_tiles[ch + 1],
                                        qt_tiles[ch + 1])
                if ch + 1 < NCH - 1:
                    rest += flat([uq[0], uk[0], uq[1], uk[1], uq[2], uk[2],
                                  uq[3], uk[3]] + uv)
                else:
                    # last chunk: only q-proj ahead of time; kt/v of the
                    # diagonal chunk become chunk-3 front fills.
                    rest += flat(uq)
                    ukv3 = flat([uk[0], uv[0], uv[1], uv[2], uv[3],
                                 uk[1], uk[2], uk[3]])
            if ch == NCH - 1 and ukv3 is not None:
                front, frate = ukv3, 6
            attention_chunk(ch, (front, frate, rest))
        for a in outproj_atoms(NCH - 1, zt_tiles[NCH - 1]):
            a()

    _split_multi_waits(nc)
    return nc


_NC_CACHE = None


def _get_nc():
    global _NC_CACHE
    if _NC_CACHE is None:
        _NC_CACHE = _build()
    return _NC_CACHE


def _make_in_maps(x, W_Q, W_K, W_V, W_O):
    x = np.asarray(x, dtype=np.float32)
    W_Q = np.asarray(W_Q, dtype=np.float32)
    W_K = np.asarray(W_K, dtype=np.float32)
    W_V = np.asarray(W_V, dtype=np.float32)
    W_O = np.asarray(W_O, dtype=np.float32)

    import ml_dtypes
    tri = np.triu(np.ones((128, 128), dtype=ml_dtypes.bfloat16))  # col >= row
    ones1 = np.ones((1, 64), dtype=np.float32)
    vones = np.ones((128, T // 128, H, 1), dtype=ml_dtypes.bfloat16)

    in_maps = []
    for core in range(8):
        b, g = core // 2, core % 2
        cs = slice(g * HG, (g + 1) * HG)
        in_maps.append({
            "xT": np.ascontiguousarray(x[b].T),
            "wq": np.ascontiguousarray(W_Q[:, cs]),
            "wk": np.ascontiguousarray(W_K[:, cs]),
            "wv": np.ascontiguousarray(W_V[:, cs]),
            "wo": np.ascontiguousarray(W_O[cs, :]),
            "tri": tri, "ones1": ones1, "vones": vones,
        })
    return in_maps


def kernel(x, W_Q, W_K, W_V, W_O):
    in_maps = _make_in_maps(x, W_Q, W_K, W_V, W_O)
    nc = _get_nc()
    res = run_bass_kernel_spmd(nc, in_maps, core_ids=list(range(8)))
    outs = [res.results[c]["out"] for c in range(8)]
    full = np.stack([outs[2 * b] + outs[2 * b + 1] for b in range(B)], axis=0)
    return full
